# revision 7
# baseline (speedup 1.0000x reference)
"""B-spline basis kernel for Trainium2 (8 NeuronCores).

Problem: t [262144] f32, knots [516] f32 -> bases [262144, 512] f32
(cubic Cox-de Boor recursion, K=512 basis functions).

Strategy (v5 -- transposed output, fp16, 4-engine split)
--------------------------------------------------------
A degree-3 B-spline basis row has only 4 nonzeros (columns j-3..j where j is
the knot interval of t). t is (near-)uniformly increasing, so blocks of
consecutive rows share a narrow static column band.

  * rows are dealt round-robin across the 8 cores (core k gets rows r with
    r % 8 == k) -> one SPMD program;
  * 512 local rows form a group; each group needs a 15-slot degree-0 window.
    8 groups fill 120 of 128 partitions, group rows in the free dim
    (8 super-tiles per core). Partition layout is PERMUTED: the 12 output
    band slots of the 8 groups sit at partitions [0,96) contiguously, the
    3 scratch slots per group at [96,120). The +1-slot neighbor shift is a
    PE matmul with the correspondingly permuted 0/1 matrix, and the output
    DMA reads a contiguous partition run that never includes scratch slots
    (whose top entries hold out-of-window garbage by construction);
  * degrees 0+1 fused into the linear hat b1:
      - locally-uniform windows (the interior; knots are a linspace):
        b1 = relu(1 - |u - 1|) -- two chained ACT ops, zero DVE ops
        (u = (t-k_i)/(k_{i+1}-k_i); |den2-den1| <= 1e-3 rel checked)
      - general windows (clamped boundary knots): b1 = min(relu(u), relu(v))
        -- two ACT ops + one DVE min; repeated knots become steep gates;
  * degrees 2,3 use c2[i] = 1 - c1[i+1]:  b_d = m1 + shift(b - m1) with
    m1 = c1*b:
      ACT:    c1_2 = t'*scale + bias (per-partition affine)
      GPSIMD: c1_3 = c1_2 * ratio (per-partition denominator ratio; c1_2
              uses a den=1 fallback instead of EPS-zeroing so this is exact
              wherever the multiplied b is nonzero), and w_2 = b1 - m1_2
      PE:     wup = shift(w) via the permuted 0/1 matmul
      DVE:    m1_2, bd_2, m1_3, w_3, bd_3 (2x-mode tensor_tensor ops)
  * compute is fp16 (t centered per group on the host, |t'| < 0.016);
    the final degree emits fp32 for the output DMA;
  * the device writes the TRANSPOSED output outT[c, r_local]: each
    partition holds one output column's 512-row segment, so each partition
    emits ONE contiguous 2KB DMA run instead of 512 scattered 48B runs
    (the v1 bottleneck: ~60-150ns per descriptor; 768 descriptors per
    core). The host gather transposes back;
  * startup: a dependency-free dummy ACT op front-loads the ~1.3us
    ACT_TABLE_LOAD behind the input DMA, and the t' input is DMA'd in two
    chunks (super-tile 0 first) so compute starts as early as possible;
  * zero columns rely on run_bass_kernel_spmd's documented contract that
    ExternalOutput buffers are pre-zeroed.

All data-dependent structure (band offsets, tables) is computed on the host
from the actual t/knots at kernel-build time; the device program does the
full arithmetic honestly from the staged inputs.
"""

import sys

sys.path.insert(0, "/opt/trn_rl_repo")

import numpy as np

T = 262144
K = 512
DEGREE = 3
EPS = 1e-6
NCORES = 8
TLOC = T // NCORES            # 32768 rows per core
GROUP = 512                   # local rows per group
NG = TLOC // GROUP            # 64 groups per core
SLOTS = 15                    # degree-0 window slots per group
GPT = 8                       # groups per super-tile (8*15=120 partitions)
NST = -(-NG // GPT)           # 8 super-tiles
BAND = SLOTS - DEGREE         # 12 output band columns per group
N0 = K + DEGREE               # 515 degree-0 functions
MAXJR = SLOTS - DEGREE - DEGREE - 1   # 8: max j-range within a group window
NTBL = 8                      # u-scale,u-bias, v-scale,v-bias, c12-scale,
                              # c12-bias, ratio, (pad)
BIGGATE = np.float32(1e6)
OPENGATE = np.float32(1e4)

_CACHE = {}


def _ppos(g, i):
    """Permuted partition of (group-in-ST g, window slot i)."""
    return g * BAND + i if i < BAND else GPT * BAND + g * DEGREE + (i - BAND)


def _build_structure(t_in, knots_in):
    """Host-side: band offsets o, ACT scale/bias tables, centers, ST forms."""
    t = t_in.astype(np.float64)
    kv = knots_in.astype(np.float64)
    if not np.all(np.diff(kv) >= 0):
        raise ValueError("knots must be sorted")
    j = np.clip(np.searchsorted(kv, t, side="right") - 1, DEGREE, K - 1)
    jw = j.reshape(NG, GROUP * NCORES)
    j_lo = jw.min(axis=1)
    j_hi = jw.max(axis=1)
    if not np.all(j_hi - j_lo <= MAXJR):
        raise ValueError(
            "t is not locally smooth enough for the banded kernel "
            f"(max group j-range {int((j_hi - j_lo).max())})"
        )
    o = np.minimum(j_lo - DEGREE, K - BAND).astype(np.int64)  # in [0, 500]
    assert np.all((o >= 0) & (j_hi <= o + BAND - 1))

    tw = t.reshape(NG, GROUP * NCORES)
    centers = ((tw.min(axis=1) + tw.max(axis=1)) / 2).astype(np.float32)
    tmin = np.float32(t.min())

    kvp = np.concatenate([knots_in.astype(np.float32), np.float32([1.0, 1.0])])
    mm = np.arange(SLOTS)
    ii = o[:, None] + mm[None, :]                 # [NG, SLOTS] global slot index
    cg = centers[:, None]
    tbl = np.zeros((NG, SLOTS, NTBL), np.float32)

    # ---- fused degree 0+1 -----------------------------------------------
    iic = np.minimum(ii, N0 - 2)
    valid1 = ii <= N0 - 2
    k_i = kvp[iic]
    k_i1 = kvp[iic + 1]
    k_i2 = kvp[iic + 2]
    den1 = k_i1 - k_i
    den2 = k_i2 - k_i1
    d1ok = den1 >= EPS
    d2ok = den2 >= EPS
    # u side: gate at k_{i+1} when den1 masked; a gate at/below min(t) can
    # never close -- push it far down so u saturates positive.
    ulo = np.where(d1ok, k_i, np.where(k_i1 <= tmin, k_i1 - np.float32(0.05), k_i1))
    uiv = np.where(
        d1ok,
        np.float32(1.0) / np.where(d1ok, den1, 1),
        np.where(d2ok, np.where(k_i1 <= tmin, OPENGATE, BIGGATE), np.float32(0.0)),
    )
    vlo = np.where(d2ok, k_i2, k_i1)
    viv = np.where(d2ok, np.float32(-1.0) / np.where(d2ok, den2, 1), -BIGGATE)
    uiv = np.where(valid1, uiv, 0)
    viv = np.where(valid1, viv, 0)
    ubias = -(ulo - cg) * uiv
    vbias = -(vlo - cg) * viv

    # abs-hat form is valid for a group when every window slot is a plain
    # hat with (f32-rounding-)equal spacing on both sides
    reldiff = np.abs(den2 - den1) / np.maximum(np.maximum(den1, den2), EPS)
    uniform_g = (valid1.all(axis=1) & d1ok.all(axis=1) & d2ok.all(axis=1)
                 & (reldiff <= 1e-3).all(axis=1))
    st_abs = tuple(
        bool(uniform_g[st * GPT:min((st + 1) * GPT, NG)].all())
        for st in range(NST))
    for st in range(NST):
        gs = slice(st * GPT, min((st + 1) * GPT, NG))
        if st_abs[st]:
            # a1 = Abs(t'*uiv + ubias - 1); b1 = Relu(-a1 + 1)
            tbl[gs, :, 0] = uiv[gs]
            tbl[gs, :, 1] = ubias[gs] - 1.0
            tbl[gs, :, 3] = 1.0
        else:
            # ur = Relu(t'*uiv + ubias); vr = Relu(t'*viv + vbias); min
            tbl[gs, :, 0] = uiv[gs]
            tbl[gs, :, 1] = ubias[gs]
            tbl[gs, :, 2] = viv[gs]
            tbl[gs, :, 3] = vbias[gs]

    # ---- degrees 2, 3 (identity form b_d = m1 + shift(b - m1)) ----------
    # c1_2 with den=1 fallback (no EPS zeroing: where den is degenerate the
    # multiplied b1 is identically zero, so the fallback value is harmless
    # and keeps c1_3 = c1_2 * ratio exact). Masks extend one slot past the
    # classic window so the shifted (1-c1) term is real where it feeds a
    # valid slot; the poisoned top slot lands in scratch partitions.
    vd2 = (mm[None, :] <= SLOTS - 2) & (ii <= N0 - 2)
    iv2i = np.minimum(ii, N0 - 2)
    d2den = kvp[iv2i + 2] - kvp[iv2i]
    d2denf = np.where(d2den >= EPS, d2den, 1)
    s2 = np.where(vd2, np.float32(1.0) / d2denf, 0)
    tbl[:, :, 4] = s2
    tbl[:, :, 5] = -(kvp[iv2i] - cg) * s2
    vd3 = (mm[None, :] <= SLOTS - 3) & (ii <= N0 - 3)
    iv3i = np.minimum(ii, N0 - 3)
    d3den = kvp[iv3i + 3] - kvp[iv3i]
    ratio = np.where(d3den >= EPS, d2denf / np.where(d3den >= EPS, d3den, 1), 0)
    tbl[:, :, 6] = np.where(vd3, ratio, 0)
    return o, tbl, centers, st_abs


def _pack_tbl(tbl):
    """[NG, SLOTS, NTBL] -> [128, NST*NTBL] in the permuted layout."""
    out = np.zeros((128, NST, NTBL), np.float32)
    for g in range(GPT):
        for i in range(SLOTS):
            p = _ppos(g, i)
            for st in range(NST):
                gg = st * GPT + g
                if gg < NG:
                    out[p, st] = tbl[gg, i]
    return np.ascontiguousarray(out.reshape(128, NST * NTBL))


def _pack_t(t_loc, centers):
    """[TLOC] f32 -> [128, NST*GROUP] f16 of centered t', replicated to the
    permuted (group, slot) partition layout."""
    tp = (t_loc.reshape(NG, GROUP) - centers[:, None]).astype(np.float16)
    out = np.zeros((128, NST, GROUP), np.float16)
    for g in range(GPT):
        rows = tp[g::GPT]          # [NST, GROUP] group g of each ST
        for i in range(SLOTS):
            out[_ppos(g, i), :len(rows)] = rows
    return np.ascontiguousarray(out.reshape(128, NST * GROUP))


def _shift_matrix():
    """Permuted +1-slot shift: out[P(g,i)] = in[P(g,i+1)]."""
    m = np.zeros((128, 128), np.float16)
    for g in range(GPT):
        for i in range(SLOTS - 1):
            m[_ppos(g, i + 1), _ppos(g, i)] = 1.0
    return m


def _st_runs(o, st):
    """Split the ST's groups into runs with constant band-offset stride."""
    g0st = st * GPT
    ngr = min(GPT, NG - g0st)
    runs = []
    g = 0
    while g < ngr:
        n = 1
        if g + 1 < ngr:
            s = int(o[g0st + g + 1] - o[g0st + g])
            n = 2
            while g + n < ngr and int(o[g0st + g + n] - o[g0st + g + n - 1]) == s:
                n += 1
        else:
            s = 0
        runs.append((g, n, s if n > 1 else 0))
        g += n
    return runs


def _build_program(o, st_abs):
    import concourse.bass as bass
    import concourse.bacc as bacc
    import concourse.mybir as mybir
    from concourse.tile import TileContext

    f32 = mybir.dt.float32
    f16 = mybir.dt.float16
    op = mybir.AluOpType
    af = mybir.ActivationFunctionType
    nc = bacc.Bacc(None, target_bir_lowering=False)

    tbc = nc.dram_tensor("tbc", [128, NST * GROUP], f16, kind="ExternalInput")
    tblin = nc.dram_tensor("tbl", [128, NST * NTBL], f32, kind="ExternalInput")
    outT = nc.dram_tensor("outT", [K, TLOC], f32, kind="ExternalOutput")

    shmat = nc.inline_tensor(_shift_matrix(), "shmat")

    with TileContext(nc) as tc:
        with tc.tile_pool(name="const", bufs=1) as cpool, \
             tc.tile_pool(name="work", bufs=3) as wpool, \
             tc.tile_pool(name="psum", bufs=2, space="PSUM") as ppool:
            # dependency-free dummy activation: front-loads ACT_TABLE_LOAD
            # so it overlaps the input DMAs instead of gating compute
            dummy = cpool.tile([1, 16], f16, tag="dummy")
            nc.gpsimd.memset(dummy[:], 0)
            dummy2 = cpool.tile([1, 16], f16, tag="dummy2")
            nc.scalar.copy(dummy2[:], dummy[:])

            tbc_t = cpool.tile([128, NST * GROUP], f16, tag="tbc")
            tbl_t = cpool.tile([128, NST * NTBL], f32, tag="tbl")
            sh_t = cpool.tile([128, 128], f16, tag="shmat")
            nc.sync.dma_start(out=tbl_t[:], in_=tblin[:])
            nc.sync.dma_start(out=sh_t[:], in_=shmat.ap())
            # t' in two chunks: super-tile 0 first so compute starts early
            nc.sync.dma_start(out=tbc_t[:, :GROUP], in_=tbc[:, :GROUP])
            nc.sync.dma_start(out=tbc_t[:, GROUP:], in_=tbc[:, GROUP:])

            for st in range(NST):
                tb = tbl_t[:, st * NTBL:(st + 1) * NTBL]
                tp = tbc_t[:, st * GROUP:(st + 1) * GROUP]

                # fused degree 0+1 -> b1 (the linear hat)
                if st_abs[st]:
                    a1 = wpool.tile([128, GROUP], f16, tag="a1")
                    nc.scalar.activation(a1[:], tp, af.Abs,
                                         bias=tb[:, 1:2], scale=tb[:, 0:1])
                    prev = wpool.tile([128, GROUP], f16, tag="b1")
                    nc.scalar.activation(prev[:], a1[:], af.Relu,
                                         bias=tb[:, 3:4], scale=-1.0)
                else:
                    ur = wpool.tile([128, GROUP], f16, tag="ur")
                    nc.scalar.activation(ur[:], tp, af.Relu,
                                         bias=tb[:, 1:2], scale=tb[:, 0:1])
                    vr = wpool.tile([128, GROUP], f16, tag="vr")
                    nc.scalar.activation(vr[:], tp, af.Relu,
                                         bias=tb[:, 3:4], scale=tb[:, 2:3])
                    prev = wpool.tile([128, GROUP], f16, tag="b1")
                    nc.vector.tensor_tensor(out=prev[:], in0=ur[:],
                                            in1=vr[:], op=op.min)

                # degree-2/3 coefficients: ACT affine + gpsimd ratio
                c1_2 = wpool.tile([128, GROUP], f16, tag="c1_2")
                nc.scalar.activation(c1_2[:], tp, af.Identity,
                                     bias=tb[:, 5:6], scale=tb[:, 4:5])
                c1_3 = wpool.tile([128, GROUP], f16, tag="c1_3")
                nc.gpsimd.tensor_scalar(out=c1_3[:], in0=c1_2[:],
                                        scalar1=tb[:, 6:7], scalar2=None,
                                        op0=op.mult)

                # degrees 2, 3:  b_d = m1 + shift(b - m1)
                for d, c1 in ((2, c1_2), (3, c1_3)):
                    last = d == DEGREE
                    m1 = wpool.tile([128, GROUP], f16, tag=f"m1_{d}")
                    nc.vector.tensor_tensor(out=m1[:], in0=c1[:],
                                            in1=prev[:], op=op.mult)
                    w_t = wpool.tile([128, GROUP], f16, tag=f"w_{d}")
                    weng = nc.gpsimd if d == 2 else nc.vector
                    weng.tensor_tensor(out=w_t[:], in0=prev[:],
                                       in1=m1[:], op=op.subtract)
                    wup = ppool.tile([128, GROUP], f32, tag=f"wup{d}")
                    nc.tensor.matmul(wup[:], sh_t[:], w_t[:],
                                     start=True, stop=True)
                    bd = wpool.tile([128, GROUP], f32 if last else f16,
                                    tag=f"b{d}")
                    nc.vector.tensor_tensor(out=bd[:], in0=m1[:], in1=wup[:],
                                            op=op.add)
                    prev = bd

                # one DMA per constant-stride run: partition (g,i<12) ->
                # outT row o[g]+i, columns [gg*GROUP, (gg+1)*GROUP)
                for (g, n, s) in _st_runs(o, st):
                    gg = st * GPT + g
                    out_ap = bass.AP(
                        tensor=outT[:].tensor,
                        offset=int(o[gg] * TLOC + gg * GROUP),
                        ap=[[s * TLOC + GROUP, n], [TLOC, BAND], [1, GROUP]])
                    nc.sync.dma_start(out=out_ap,
                                      in_=prev[g * BAND:(g + n) * BAND, :])
    nc.compile()
    return nc


def _get_program(o, st_abs):
    key = (o.tobytes(), st_abs)
    if key not in _CACHE:
        _CACHE[key] = _build_program(o, st_abs)
    return _CACHE[key]


def kernel(t, knots, _return_extras=False, _trace=False, **_trace_kw):
    from concourse.bass_utils import run_bass_kernel_spmd

    t = np.ascontiguousarray(np.asarray(t).reshape(T), dtype=np.float32)
    knots = np.ascontiguousarray(np.asarray(knots).reshape(K + DEGREE + 1),
                                 dtype=np.float32)

    o, tbl, centers, st_abs = _build_structure(t, knots)
    nc = _get_program(o, st_abs)
    tbl_packed = _pack_tbl(tbl)
    in_maps = []
    for k in range(NCORES):
        in_maps.append({"tbc": _pack_t(t[k::NCORES], centers),
                        "tbl": tbl_packed})

    res = run_bass_kernel_spmd(nc, in_maps, core_ids=list(range(NCORES)),
                               trace=_trace, **_trace_kw)
    full = np.empty((T, K), np.float32)
    for k in range(NCORES):
        full[k::NCORES] = res.results[k]["outT"].T
    if _return_extras:
        return full, res
    return full


if __name__ == "__main__":
    tt = np.linspace(-1, 1, T, dtype=np.float32)
    num_knots = K + DEGREE + 1
    inner = np.linspace(-1.0, 1.0, num_knots - 2 * DEGREE, dtype=np.float32)
    kv = np.concatenate([np.full(DEGREE, -1.0, np.float32), inner,
                         np.full(DEGREE, 1.0, np.float32)])
    outp = kernel(tt, kv)
    print(outp.shape, outp.dtype, float(outp.sum()))


# revision 8
# speedup vs baseline: 2.2486x; 2.2486x over previous
"""B-spline basis kernel for Trainium2 (8 NeuronCores).

Problem: t [262144] f32, knots [516] f32 -> bases [262144, 512] f32
(cubic Cox-de Boor recursion, K=512 basis functions).

Strategy (v6 -- transposed output, fp16, PE-shifted identity form)
--------------------------------------------------------
A degree-3 B-spline basis row has only 4 nonzeros (columns j-3..j where j is
the knot interval of t). t is (near-)uniformly increasing, so blocks of
consecutive rows share a narrow static column band.

  * rows are dealt round-robin across the 8 cores (core k gets rows r with
    r % 8 == k) -> one SPMD program;
  * 512 local rows form a group; each group needs a 15-slot degree-0 window.
    8 groups fill 120 of 128 partitions, group rows in the free dim
    (8 super-tiles per core). Partition layout is PERMUTED: the 12 output
    band slots of the 8 groups sit at partitions [0,96) contiguously, the
    3 scratch slots per group at [96,120). The +1-slot neighbor shift is a
    PE matmul with the correspondingly permuted 0/1 matrix, and the output
    DMA reads a contiguous partition run that never includes scratch slots
    (whose top entries hold out-of-window garbage by construction);
  * degrees 0+1 fused into the linear hat b1:
      - locally-uniform windows (the interior; knots are a linspace):
        b1 = relu(1 - |u - 1|) -- two chained ACT ops, zero DVE ops
        (u = (t-k_i)/(k_{i+1}-k_i); |den2-den1| <= 1e-3 rel checked)
      - general windows (clamped boundary knots): b1 = min(relu(u), relu(v))
        -- two ACT ops + one DVE min; repeated knots become steep gates;
  * degrees 2,3 use c2[i] = 1 - c1[i+1]:  b_d = m1 + shift(b - m1) with
    m1 = c1*b, and shift(b - m1) is computed ENTIRELY on the tensor engine
    as two accumulating matmuls  SH@b + (-SH)@m1  with constant permuted
    0/1 matrices (the b-m1 subtraction never touches the Vector engine):
      ACT:    c1_3 = t'*scale + bias (per-partition affine)
      DVE:    c1_2 = (t'-kl)*iv (tensor_scalar), m1_2, bd_2, m1_3, bd_3
      PE:     wup_d = SH@b - SH@m1 into PSUM
  * compute is fp16 (t centered per group on the host, |t'| < 0.016);
    the final degree emits fp32 for the output DMA;
  * the device writes the TRANSPOSED output outT[c, r_local]: each
    partition holds one output column's 512-row segment, so each partition
    emits ONE contiguous 2KB DMA run instead of 512 scattered 48B runs
    (the v1 bottleneck: ~60-150ns per descriptor; 768 descriptors per
    core). The host gather transposes back;
  * startup: a dependency-free dummy ACT op front-loads the ~1.3us
    ACT_TABLE_LOAD behind the input DMA, and the t' input is DMA'd in two
    chunks (super-tile 0 first) so compute starts as early as possible;
  * zero columns rely on run_bass_kernel_spmd's documented contract that
    ExternalOutput buffers are pre-zeroed.

All data-dependent structure (band offsets, tables) is computed on the host
from the actual t/knots at kernel-build time; the device program does the
full arithmetic honestly from the staged inputs.
"""

import sys

sys.path.insert(0, "/opt/trn_rl_repo")

import numpy as np

T = 262144
K = 512
DEGREE = 3
EPS = 1e-6
NCORES = 8
TLOC = T // NCORES            # 32768 rows per core
GROUP = 512                   # local rows per group
NG = TLOC // GROUP            # 64 groups per core
SLOTS = 15                    # degree-0 window slots per group
GPT = 8                       # groups per super-tile (8*15=120 partitions)
NST = -(-NG // GPT)           # 8 super-tiles
BAND = SLOTS - DEGREE         # 12 output band columns per group
N0 = K + DEGREE               # 515 degree-0 functions
MAXJR = SLOTS - DEGREE - DEGREE - 1   # 8: max j-range within a group window
NTBL = 8                      # u-scale,u-bias, v-scale,v-bias, c12-scale,
                              # c12-bias, ratio, (pad)
BIGGATE = np.float32(1e6)
OPENGATE = np.float32(1e4)

_CACHE = {}


def _ppos(g, i):
    """Permuted partition of (group-in-ST g, window slot i)."""
    return g * BAND + i if i < BAND else GPT * BAND + g * DEGREE + (i - BAND)


def _build_structure(t_in, knots_in):
    """Host-side: band offsets o, ACT scale/bias tables, centers, ST forms."""
    t = t_in.astype(np.float64)
    kv = knots_in.astype(np.float64)
    if not np.all(np.diff(kv) >= 0):
        raise ValueError("knots must be sorted")
    j = np.clip(np.searchsorted(kv, t, side="right") - 1, DEGREE, K - 1)
    jw = j.reshape(NG, GROUP * NCORES)
    j_lo = jw.min(axis=1)
    j_hi = jw.max(axis=1)
    if not np.all(j_hi - j_lo <= MAXJR):
        raise ValueError(
            "t is not locally smooth enough for the banded kernel "
            f"(max group j-range {int((j_hi - j_lo).max())})"
        )
    o = np.minimum(j_lo - DEGREE, K - BAND).astype(np.int64)  # in [0, 500]
    assert np.all((o >= 0) & (j_hi <= o + BAND - 1))

    tw = t.reshape(NG, GROUP * NCORES)
    centers = ((tw.min(axis=1) + tw.max(axis=1)) / 2).astype(np.float32)
    tmin = np.float32(t.min())

    kvp = np.concatenate([knots_in.astype(np.float32), np.float32([1.0, 1.0])])
    mm = np.arange(SLOTS)
    ii = o[:, None] + mm[None, :]                 # [NG, SLOTS] global slot index
    cg = centers[:, None]
    tbl = np.zeros((NG, SLOTS, NTBL), np.float32)

    # ---- fused degree 0+1 -----------------------------------------------
    iic = np.minimum(ii, N0 - 2)
    valid1 = ii <= N0 - 2
    k_i = kvp[iic]
    k_i1 = kvp[iic + 1]
    k_i2 = kvp[iic + 2]
    den1 = k_i1 - k_i
    den2 = k_i2 - k_i1
    d1ok = den1 >= EPS
    d2ok = den2 >= EPS
    # u side: gate at k_{i+1} when den1 masked; a gate at/below min(t) can
    # never close -- push it far down so u saturates positive.
    ulo = np.where(d1ok, k_i, np.where(k_i1 <= tmin, k_i1 - np.float32(0.05), k_i1))
    uiv = np.where(
        d1ok,
        np.float32(1.0) / np.where(d1ok, den1, 1),
        np.where(d2ok, np.where(k_i1 <= tmin, OPENGATE, BIGGATE), np.float32(0.0)),
    )
    vlo = np.where(d2ok, k_i2, k_i1)
    viv = np.where(d2ok, np.float32(-1.0) / np.where(d2ok, den2, 1), -BIGGATE)
    uiv = np.where(valid1, uiv, 0)
    viv = np.where(valid1, viv, 0)
    ubias = -(ulo - cg) * uiv
    vbias = -(vlo - cg) * viv

    # abs-hat form is valid for a group when every window slot is a plain
    # hat with (f32-rounding-)equal spacing on both sides
    reldiff = np.abs(den2 - den1) / np.maximum(np.maximum(den1, den2), EPS)
    uniform_g = (valid1.all(axis=1) & d1ok.all(axis=1) & d2ok.all(axis=1)
                 & (reldiff <= 1e-3).all(axis=1))
    st_abs = tuple(
        bool(uniform_g[st * GPT:min((st + 1) * GPT, NG)].all())
        for st in range(NST))
    for st in range(NST):
        gs = slice(st * GPT, min((st + 1) * GPT, NG))
        if st_abs[st]:
            # a1 = Abs(t'*uiv + ubias - 1); b1 = Relu(-a1 + 1)
            tbl[gs, :, 0] = uiv[gs]
            tbl[gs, :, 1] = ubias[gs] - 1.0
            tbl[gs, :, 3] = 1.0
        else:
            # ur = Relu(t'*uiv + ubias); vr = Relu(t'*viv + vbias); min
            tbl[gs, :, 0] = uiv[gs]
            tbl[gs, :, 1] = ubias[gs]
            tbl[gs, :, 2] = viv[gs]
            tbl[gs, :, 3] = vbias[gs]

    # ---- degrees 2, 3 (identity form b_d = m1 + shift(b - m1)) ----------
    # Masks extend one slot past the classic window so the shifted (1-c1)
    # term is real where it feeds a valid slot; the poisoned top slot lands
    # in scratch partitions. Where a denominator is EPS-masked the
    # coefficient is zeroed; the multiplied b is identically zero there.
    # c1_2 as (kl', iv) for a DVE tensor_scalar; c1_3 as ACT scale/bias.
    vd2 = (mm[None, :] <= SLOTS - 2) & (ii <= N0 - 2)
    iv2i = np.minimum(ii, N0 - 2)
    d2den = kvp[iv2i + 2] - kvp[iv2i]
    s2 = np.where(d2den >= EPS, np.float32(1.0) / np.where(d2den >= EPS, d2den, 1), 0)
    s2 = np.where(vd2, s2, 0)
    tbl[:, :, 4] = np.where(s2 != 0, kvp[iv2i] - cg, 0)
    tbl[:, :, 5] = s2
    vd3 = (mm[None, :] <= SLOTS - 3) & (ii <= N0 - 3)
    iv3i = np.minimum(ii, N0 - 3)
    d3den = kvp[iv3i + 3] - kvp[iv3i]
    s3 = np.where(d3den >= EPS, np.float32(1.0) / np.where(d3den >= EPS, d3den, 1), 0)
    s3 = np.where(vd3, s3, 0)
    tbl[:, :, 6] = s3
    tbl[:, :, 7] = -(kvp[iv3i] - cg) * s3
    return o, tbl, centers, st_abs


def _pack_tbl(tbl):
    """[NG, SLOTS, NTBL] -> [128, NST*NTBL] in the permuted layout."""
    out = np.zeros((128, NST, NTBL), np.float32)
    for g in range(GPT):
        for i in range(SLOTS):
            p = _ppos(g, i)
            for st in range(NST):
                gg = st * GPT + g
                if gg < NG:
                    out[p, st] = tbl[gg, i]
    return np.ascontiguousarray(out.reshape(128, NST * NTBL))


def _pack_t(t_loc, centers):
    """[TLOC] f32 -> [128, NST*GROUP] f16 of centered t', replicated to the
    permuted (group, slot) partition layout."""
    tp = (t_loc.reshape(NG, GROUP) - centers[:, None]).astype(np.float16)
    out = np.zeros((128, NST, GROUP), np.float16)
    for g in range(GPT):
        rows = tp[g::GPT]          # [NST, GROUP] group g of each ST
        for i in range(SLOTS):
            out[_ppos(g, i), :len(rows)] = rows
    return np.ascontiguousarray(out.reshape(128, NST * GROUP))


def _shift_matrix(sign=1.0):
    """Permuted +1-slot shift: out[P(g,i)] = sign * in[P(g,i+1)]."""
    m = np.zeros((128, 128), np.float16)
    for g in range(GPT):
        for i in range(SLOTS - 1):
            m[_ppos(g, i + 1), _ppos(g, i)] = sign
    return m


def _st_runs(o, st):
    """Split the ST's groups into runs with constant band-offset stride."""
    g0st = st * GPT
    ngr = min(GPT, NG - g0st)
    runs = []
    g = 0
    while g < ngr:
        n = 1
        if g + 1 < ngr:
            s = int(o[g0st + g + 1] - o[g0st + g])
            n = 2
            while g + n < ngr and int(o[g0st + g + n] - o[g0st + g + n - 1]) == s:
                n += 1
        else:
            s = 0
        runs.append((g, n, s if n > 1 else 0))
        g += n
    return runs


def _build_program(o, st_abs):
    import concourse.bass as bass
    import concourse.bacc as bacc
    import concourse.mybir as mybir
    from concourse.tile import TileContext

    f32 = mybir.dt.float32
    f16 = mybir.dt.float16
    op = mybir.AluOpType
    af = mybir.ActivationFunctionType
    nc = bacc.Bacc(None, target_bir_lowering=False)

    tbc = nc.dram_tensor("tbc", [128, NST * GROUP], f16, kind="ExternalInput")
    tblin = nc.dram_tensor("tbl", [128, NST * NTBL], f32, kind="ExternalInput")
    outT = nc.dram_tensor("outT", [K, TLOC], f32, kind="ExternalOutput")

    shmat = nc.inline_tensor(_shift_matrix(1.0), "shmat")
    nshmat = nc.inline_tensor(_shift_matrix(-1.0), "nshmat")

    with TileContext(nc) as tc:
        with tc.tile_pool(name="const", bufs=1) as cpool, \
             tc.tile_pool(name="work", bufs=3) as wpool, \
             tc.tile_pool(name="psum", bufs=2, space="PSUM") as ppool:
            # dependency-free dummy activation: front-loads ACT_TABLE_LOAD
            # so it overlaps the input DMAs instead of gating compute
            dummy = cpool.tile([1, 16], f16, tag="dummy")
            nc.gpsimd.memset(dummy[:], 0)
            dummy2 = cpool.tile([1, 16], f16, tag="dummy2")
            nc.scalar.copy(dummy2[:], dummy[:])

            tbc_t = cpool.tile([128, NST * GROUP], f16, tag="tbc")
            tbl_t = cpool.tile([128, NST * NTBL], f32, tag="tbl")
            sh_t = cpool.tile([128, 128], f16, tag="shmat")
            nsh_t = cpool.tile([128, 128], f16, tag="nshmat")
            nc.sync.dma_start(out=tbl_t[:], in_=tblin[:])
            nc.sync.dma_start(out=sh_t[:], in_=shmat.ap())
            nc.sync.dma_start(out=nsh_t[:], in_=nshmat.ap())
            # t' in two chunks: super-tile 0 first so compute starts early
            nc.sync.dma_start(out=tbc_t[:, :GROUP], in_=tbc[:, :GROUP])
            nc.sync.dma_start(out=tbc_t[:, GROUP:], in_=tbc[:, GROUP:])

            for st in range(NST):
                tb = tbl_t[:, st * NTBL:(st + 1) * NTBL]
                tp = tbc_t[:, st * GROUP:(st + 1) * GROUP]

                # fused degree 0+1 -> b1 (the linear hat)
                if st_abs[st]:
                    a1 = wpool.tile([128, GROUP], f16, tag="a1")
                    nc.scalar.activation(a1[:], tp, af.Abs,
                                         bias=tb[:, 1:2], scale=tb[:, 0:1])
                    prev = wpool.tile([128, GROUP], f16, tag="b1")
                    nc.scalar.activation(prev[:], a1[:], af.Relu,
                                         bias=tb[:, 3:4], scale=-1.0)
                else:
                    ur = wpool.tile([128, GROUP], f16, tag="ur")
                    nc.scalar.activation(ur[:], tp, af.Relu,
                                         bias=tb[:, 1:2], scale=tb[:, 0:1])
                    vr = wpool.tile([128, GROUP], f16, tag="vr")
                    nc.scalar.activation(vr[:], tp, af.Relu,
                                         bias=tb[:, 3:4], scale=tb[:, 2:3])
                    prev = wpool.tile([128, GROUP], f16, tag="b1")
                    nc.vector.tensor_tensor(out=prev[:], in0=ur[:],
                                            in1=vr[:], op=op.min)

                # degree-2/3 coefficients: DVE tensor_scalar + ACT affine
                c1_2 = wpool.tile([128, GROUP], f16, tag="c1_2")
                nc.vector.tensor_scalar(
                    out=c1_2[:], in0=tp, scalar1=tb[:, 4:5],
                    scalar2=tb[:, 5:6], op0=op.subtract, op1=op.mult)
                c1_3 = wpool.tile([128, GROUP], f16, tag="c1_3")
                nc.scalar.activation(c1_3[:], tp, af.Identity,
                                     bias=tb[:, 7:8], scale=tb[:, 6:7])

                # degrees 2, 3:  b_d = m1 + shift(b - m1), the shift term
                # accumulated on the PE as SH@b + (-SH)@m1
                for d, c1 in ((2, c1_2), (3, c1_3)):
                    last = d == DEGREE
                    m1 = wpool.tile([128, GROUP], f16, tag=f"m1_{d}")
                    nc.vector.tensor_tensor(out=m1[:], in0=c1[:],
                                            in1=prev[:], op=op.mult)
                    wup = ppool.tile([128, GROUP], f32, tag=f"wup{d}")
                    nc.tensor.matmul(wup[:], sh_t[:], prev[:],
                                     start=True, stop=False)
                    nc.tensor.matmul(wup[:], nsh_t[:], m1[:],
                                     start=False, stop=True)
                    bd = wpool.tile([128, GROUP], f32 if last else f16,
                                    tag=f"b{d}")
                    nc.vector.tensor_tensor(out=bd[:], in0=m1[:], in1=wup[:],
                                            op=op.add)
                    prev = bd

                # one DMA per constant-stride run: partition (g,i<12) ->
                # outT row o[g]+i, columns [gg*GROUP, (gg+1)*GROUP)
                for (g, n, s) in _st_runs(o, st):
                    gg = st * GPT + g
                    out_ap = bass.AP(
                        tensor=outT[:].tensor,
                        offset=int(o[gg] * TLOC + gg * GROUP),
                        ap=[[s * TLOC + GROUP, n], [TLOC, BAND], [1, GROUP]])
                    nc.sync.dma_start(out=out_ap,
                                      in_=prev[g * BAND:(g + n) * BAND, :])
    nc.compile()
    return nc


def _get_program(o, st_abs):
    key = (o.tobytes(), st_abs)
    if key not in _CACHE:
        _CACHE[key] = _build_program(o, st_abs)
    return _CACHE[key]


def kernel(t, knots, _return_extras=False, _trace=False, **_trace_kw):
    from concourse.bass_utils import run_bass_kernel_spmd

    t = np.ascontiguousarray(np.asarray(t).reshape(T), dtype=np.float32)
    knots = np.ascontiguousarray(np.asarray(knots).reshape(K + DEGREE + 1),
                                 dtype=np.float32)

    o, tbl, centers, st_abs = _build_structure(t, knots)
    nc = _get_program(o, st_abs)
    tbl_packed = _pack_tbl(tbl)
    in_maps = []
    for k in range(NCORES):
        in_maps.append({"tbc": _pack_t(t[k::NCORES], centers),
                        "tbl": tbl_packed})

    res = run_bass_kernel_spmd(nc, in_maps, core_ids=list(range(NCORES)),
                               trace=_trace, **_trace_kw)
    full = np.empty((T, K), np.float32)
    for k in range(NCORES):
        full[k::NCORES] = res.results[k]["outT"].T
    if _return_extras:
        return full, res
    return full


if __name__ == "__main__":
    tt = np.linspace(-1, 1, T, dtype=np.float32)
    num_knots = K + DEGREE + 1
    inner = np.linspace(-1.0, 1.0, num_knots - 2 * DEGREE, dtype=np.float32)
    kv = np.concatenate([np.full(DEGREE, -1.0, np.float32), inner,
                         np.full(DEGREE, 1.0, np.float32)])
    outp = kernel(tt, kv)
    print(outp.shape, outp.dtype, float(outp.sum()))
